# revision 1
# baseline (speedup 1.0000x reference)
"""VQ codebook kernel (nn_ApplyKmeans): dist = ||x||^2 - 2 x@C + Cnorm; argmin; gather.

Strategy (8 NeuronCores, data-parallel over rows of x):
  - Host: shard x by rows; pre-transpose each shard to [D, Nsh] and pre-round
    x and C to the float32r grid (RNE, 11 mantissa bits) so the TensorEngine
    can run fp32r matmuls at bf16 speed (1 cycle/row).
  - Device per core: scores = xT.T @ C (fp32r, fp32 accumulate in PSUM),
    subtract Cnorm/2 (DVE), top-8 max + max_index (DVE), indirect-DMA gather
    of codewords from C.T rows, DMA out. Top-2 score values are exported so
    the host can re-check rows where the argmax margin is below the fp32r
    noise floor.
  - Host: exact (float64) re-scoring of the few low-margin rows.
"""

import sys

sys.path.insert(0, "/opt/trn_rl_repo")

import numpy as np

import concourse.bass as bass
import concourse.mybir as mybir
from concourse import bacc
from concourse.tile import TileContext
from concourse.bass_utils import run_bass_kernel_spmd

N, D, K = 262144, 768, 1024
NCORES = 8
NSH = N // NCORES            # 32768 rows per core
DCH = D // 128               # 6 contraction chunks
MT = 512                     # rows per DMA tile
NOT = NSH // MT              # 64 outer tiles
NST = NSH // 128             # 256 sub-tiles of 128 rows
GAP_THETA = 0.05             # host re-check margin threshold


def rne11(a: np.ndarray) -> np.ndarray:
    """Round f32 array to the float32r grid: round-to-nearest-even keeping
    11 explicit mantissa bits (verified on hardware via DVE round-trip)."""
    u = a.view(np.uint32)
    out = np.empty_like(u)
    CH = 1 << 22
    flat_in = u.reshape(-1)
    flat_out = out.reshape(-1)
    for i in range(0, flat_in.size, CH):
        v = flat_in[i:i + CH].astype(np.uint64)
        lsb = (v >> np.uint64(12)) & np.uint64(1)
        v = (v + np.uint64(0x7FF) + lsb) & np.uint64(0xFFFFF000)
        flat_out[i:i + CH] = v.astype(np.uint32)
    return out.view(np.float32).reshape(a.shape)


def build_kernel():
    nc = bacc.Bacc()
    xt_ext = nc.declare_dram_parameter("xt", [D, NSH], mybir.dt.float32r, isOutput=False)
    cb_ext = nc.declare_dram_parameter("cb", [D, K], mybir.dt.float32r, isOutput=False)
    cnb_ext = nc.declare_dram_parameter("cnb", [128, K], mybir.dt.float32, isOutput=False)
    ct_ext = nc.declare_dram_parameter("ct", [K, D], mybir.dt.float32, isOutput=False)
    out_ext = nc.declare_dram_parameter("out", [NSH, D], mybir.dt.float32, isOutput=True)
    mx_ext = nc.declare_dram_parameter("mx", [128, NST * 8], mybir.dt.float32, isOutput=True)

    with TileContext(nc) as tc:
        with (
            tc.tile_pool(name="const", bufs=1) as const_pool,
            tc.tile_pool(name="xp", bufs=3) as xpool,
            tc.tile_pool(name="scp", bufs=3) as scpool,
            tc.tile_pool(name="cwp", bufs=4) as cwpool,
            tc.tile_pool(name="small", bufs=4) as smpool,
            tc.tile_pool(name="ps", bufs=3, space="PSUM") as pspool,
        ):
            csb = const_pool.tile([128, DCH, K], mybir.dt.float32r)
            nc.sync.dma_start(out=csb[:], in_=cb_ext[:].rearrange("(c p) k -> p c k", p=128))
            cnb = const_pool.tile([128, K], mybir.dt.float32)
            nc.sync.dma_start(out=cnb[:], in_=cnb_ext[:])
            gbuf = const_pool.tile([128, NST * 8], mybir.dt.float32)

            for ot in range(NOT):
                xtile = xpool.tile([128, DCH, MT], mybir.dt.float32r, tag="xt")
                nc.sync.dma_start(
                    out=xtile[:],
                    in_=xt_ext[:, ot * MT:(ot + 1) * MT].rearrange("(c p) m -> p c m", p=128),
                )
                for st in range(MT // 128):
                    t = ot * (MT // 128) + st
                    psum = pspool.tile([128, K], mybir.dt.float32, space="PSUM", tag="ps")
                    for d in range(DCH):
                        for h in range(2):
                            nc.tensor.matmul(
                                out=psum[:, h * 512:(h + 1) * 512],
                                lhsT=xtile[:, d, st * 128:(st + 1) * 128],
                                rhs=csb[:, d, h * 512:(h + 1) * 512],
                                start=(d == 0),
                                stop=(d == DCH - 1),
                            )
                    scores = scpool.tile([128, K], mybir.dt.float32, tag="sc")
                    nc.vector.tensor_tensor(
                        out=scores[:], in0=psum[:], in1=cnb[:], op=mybir.AluOpType.subtract
                    )
                    mx8 = gbuf[:, t * 8:(t + 1) * 8]
                    nc.vector.max(out=mx8, in_=scores[:])
                    idx8 = smpool.tile([128, 8], mybir.dt.uint32, tag="idx")
                    nc.vector.max_index(out=idx8[:], in_max=mx8, in_values=scores[:])
                    cw = cwpool.tile([128, D], mybir.dt.float32, tag="cw")
                    nc.gpsimd.indirect_dma_start(
                        out=cw[:],
                        out_offset=None,
                        in_=ct_ext[:],
                        in_offset=bass.IndirectOffsetOnAxis(ap=idx8[:, :1], axis=0),
                    )
                    nc.sync.dma_start(out=out_ext[t * 128:(t + 1) * 128, :], in_=cw[:])
            nc.sync.dma_start(out=mx_ext[:], in_=gbuf[:])

    nc.finalize()
    return nc


def _prep_core(args):
    x, c = args
    xs = x[c * NSH:(c + 1) * NSH]
    return rne11(np.ascontiguousarray(xs.T))


def kernel(x, C, Cnorm):
    x = np.ascontiguousarray(np.asarray(x, dtype=np.float32))
    C = np.ascontiguousarray(np.asarray(C, dtype=np.float32))
    Cnorm = np.asarray(Cnorm, dtype=np.float32).reshape(1, K)

    from concurrent.futures import ThreadPoolExecutor
    with ThreadPoolExecutor(max_workers=8) as ex:
        xts = list(ex.map(_prep_core, [(x, c) for c in range(NCORES)]))

    cb = rne11(C)
    cnb = np.ascontiguousarray(np.broadcast_to(Cnorm * 0.5, (128, K)).astype(np.float32))
    ct = np.ascontiguousarray(C.T)

    in_maps = [{"xt": xts[c], "cb": cb, "cnb": cnb, "ct": ct} for c in range(NCORES)]

    nc = build_kernel()
    res = run_bass_kernel_spmd(nc, in_maps, core_ids=list(range(NCORES))).results

    out = np.empty((N, D), dtype=np.float32)
    recheck_rows = []
    for c in range(NCORES):
        out[c * NSH:(c + 1) * NSH] = res[c]["out"]
        mx = res[c]["mx"].reshape(128, NST, 8)
        gap = mx[:, :, 0] - mx[:, :, 1]          # [partition, subtile]
        gap_rows = gap.T.reshape(-1)             # shard row = t*128 + p
        rows = np.nonzero(gap_rows < GAP_THETA)[0]
        if rows.size:
            recheck_rows.append(rows + c * NSH)

    if recheck_rows:
        rows = np.concatenate(recheck_rows)
        xr = x[rows].astype(np.float64)
        dist = (
            np.sum(xr * xr, axis=1, keepdims=True)
            - 2.0 * (xr @ C.astype(np.float64))
            + Cnorm.astype(np.float64)
        )
        ids = np.argmin(dist, axis=1)
        out[rows] = C.T[ids]

    return out


# revision 2
# speedup vs baseline: 1.0438x; 1.0438x over previous
"""VQ codebook kernel (nn_ApplyKmeans): dist = ||x||^2 - 2 x@C + Cnorm; argmin; gather.

Strategy (8 NeuronCores, data-parallel over rows of x):
  - Host: shard x by rows; pre-transpose each shard to [D, Nsh] and cast to
    fp16 (halves the input DMA; the TensorEngine runs fp16 matmuls at full
    rate with fp32 accumulation in PSUM).
  - Device per core: scores = xT.T @ C (fp16 in, fp32 accumulate),
    subtract Cnorm/2 (DVE), top-8 max + max_index (DVE), indirect-DMA gather
    of codewords from C.T rows (fp32, exact), DMA out. Top-2 score values
    are exported so the host can re-check rows where the argmax margin is
    below the fp16 noise floor.
  - Host: exact (float64) re-scoring of the few low-margin rows.
"""

import sys

sys.path.insert(0, "/opt/trn_rl_repo")

import numpy as np

import concourse.bass as bass
import concourse.mybir as mybir
from concourse import bacc
from concourse.tile import TileContext
from concourse.bass_utils import run_bass_kernel_spmd

N, D, K = 262144, 768, 1024
NCORES = 8
NSH = N // NCORES            # 32768 rows per core
DCH = D // 128               # 6 contraction chunks
MT = 512                     # rows per DMA tile
NOT = NSH // MT              # 64 outer tiles
NST = NSH // 128             # 256 sub-tiles of 128 rows
GAP_THETA = 0.10             # host re-check margin threshold (fp16 noise ~0.009)

COMPUTE_DT = mybir.dt.float16
COMPUTE_NP = np.float16


def build_kernel():
    nc = bacc.Bacc()
    xt_ext = nc.declare_dram_parameter("xt", [D, NSH], COMPUTE_DT, isOutput=False)
    cb_ext = nc.declare_dram_parameter("cb", [D, K], COMPUTE_DT, isOutput=False)
    cnb_ext = nc.declare_dram_parameter("cnb", [128, K], mybir.dt.float32, isOutput=False)
    ct_ext = nc.declare_dram_parameter("ct", [K, D], mybir.dt.float32, isOutput=False)
    out_ext = nc.declare_dram_parameter("out", [NSH, D], mybir.dt.float32, isOutput=True)
    mx_ext = nc.declare_dram_parameter("mx", [128, NST * 8], mybir.dt.float32, isOutput=True)

    with TileContext(nc) as tc:
        with (
            tc.tile_pool(name="const", bufs=1) as const_pool,
            tc.tile_pool(name="xp", bufs=3) as xpool,
            tc.tile_pool(name="scp", bufs=3) as scpool,
            tc.tile_pool(name="cwp", bufs=4) as cwpool,
            tc.tile_pool(name="small", bufs=4) as smpool,
            tc.tile_pool(name="ps", bufs=3, space="PSUM") as pspool,
        ):
            csb = const_pool.tile([128, DCH, K], COMPUTE_DT)
            nc.sync.dma_start(out=csb[:], in_=cb_ext[:].rearrange("(c p) k -> p c k", p=128))
            cnb = const_pool.tile([128, K], mybir.dt.float32)
            nc.sync.dma_start(out=cnb[:], in_=cnb_ext[:])
            gbuf = const_pool.tile([128, NST * 8], mybir.dt.float32)

            for ot in range(NOT):
                xtile = xpool.tile([128, DCH, MT], COMPUTE_DT, tag="xt")
                nc.sync.dma_start(
                    out=xtile[:],
                    in_=xt_ext[:, ot * MT:(ot + 1) * MT].rearrange("(c p) m -> p c m", p=128),
                )
                for st in range(MT // 128):
                    t = ot * (MT // 128) + st
                    psum = pspool.tile([128, K], mybir.dt.float32, space="PSUM", tag="ps")
                    for d in range(DCH):
                        for h in range(2):
                            nc.tensor.matmul(
                                out=psum[:, h * 512:(h + 1) * 512],
                                lhsT=xtile[:, d, st * 128:(st + 1) * 128],
                                rhs=csb[:, d, h * 512:(h + 1) * 512],
                                start=(d == 0),
                                stop=(d == DCH - 1),
                            )
                    scores = scpool.tile([128, K], mybir.dt.float32, tag="sc")
                    nc.vector.tensor_tensor(
                        out=scores[:], in0=psum[:], in1=cnb[:], op=mybir.AluOpType.subtract
                    )
                    mx8 = gbuf[:, t * 8:(t + 1) * 8]
                    nc.vector.max(out=mx8, in_=scores[:])
                    idx8 = smpool.tile([128, 8], mybir.dt.uint32, tag="idx")
                    nc.vector.max_index(out=idx8[:], in_max=mx8, in_values=scores[:])
                    cw = cwpool.tile([128, D], mybir.dt.float32, tag="cw")
                    nc.gpsimd.indirect_dma_start(
                        out=cw[:],
                        out_offset=None,
                        in_=ct_ext[:],
                        in_offset=bass.IndirectOffsetOnAxis(ap=idx8[:, :1], axis=0),
                    )
                    nc.sync.dma_start(out=out_ext[t * 128:(t + 1) * 128, :], in_=cw[:])
            nc.sync.dma_start(out=mx_ext[:], in_=gbuf[:])

    nc.finalize()
    return nc


def _prep_core(args):
    x, c = args
    xs = x[c * NSH:(c + 1) * NSH]
    return np.ascontiguousarray(xs.T.astype(COMPUTE_NP))


def prepare_in_maps(x, C, Cnorm):
    x = np.ascontiguousarray(np.asarray(x, dtype=np.float32))
    C = np.ascontiguousarray(np.asarray(C, dtype=np.float32))
    Cnorm = np.asarray(Cnorm, dtype=np.float32).reshape(1, K)

    from concurrent.futures import ThreadPoolExecutor
    with ThreadPoolExecutor(max_workers=8) as ex:
        xts = list(ex.map(_prep_core, [(x, c) for c in range(NCORES)]))

    cb = C.astype(COMPUTE_NP)
    cnb = np.ascontiguousarray(np.broadcast_to(Cnorm * 0.5, (128, K)).astype(np.float32))
    ct = np.ascontiguousarray(C.T)
    return [{"xt": xts[c], "cb": cb, "cnb": cnb, "ct": ct} for c in range(NCORES)]


def postprocess(results, x, C, Cnorm):
    """Assemble shard outputs and exactly re-score low-margin rows."""
    x = np.asarray(x, dtype=np.float32)
    C = np.asarray(C, dtype=np.float32)
    Cnorm = np.asarray(Cnorm, dtype=np.float32).reshape(1, K)
    out = np.empty((N, D), dtype=np.float32)
    recheck_rows = []
    for c in range(NCORES):
        out[c * NSH:(c + 1) * NSH] = results[c]["out"]
        mx = results[c]["mx"].reshape(128, NST, 8)
        gap = mx[:, :, 0] - mx[:, :, 1]          # [partition, subtile]
        gap_rows = gap.T.reshape(-1)             # shard row = t*128 + p
        rows = np.nonzero(gap_rows < GAP_THETA)[0]
        if rows.size:
            recheck_rows.append(rows + c * NSH)

    if recheck_rows:
        rows = np.concatenate(recheck_rows)
        xr = x[rows].astype(np.float64)
        dist = (
            np.sum(xr * xr, axis=1, keepdims=True)
            - 2.0 * (xr @ C.astype(np.float64))
            + Cnorm.astype(np.float64)
        )
        ids = np.argmin(dist, axis=1)
        out[rows] = C.T[ids]
    return out


def kernel(x, C, Cnorm):
    in_maps = prepare_in_maps(x, C, Cnorm)
    nc = build_kernel()
    res = run_bass_kernel_spmd(nc, in_maps, core_ids=list(range(NCORES))).results
    return postprocess(res, x, C, Cnorm)
